# revision 1
# baseline (speedup 1.0000x reference)
"""AttentionHead kernel for Trainium2, 8 NeuronCores, data-parallel over batch.

Problem (fixed shapes):
    input_tensor [8, 2048, 1024] f32, attention_mask [8, 2048] int64 (0/1),
    Wq/Wk/Wv [1024, 128] f32, bq/bk/bv [128] f32.
    out = softmax(mask(Q @ K^T / sqrt(2048))) @ V    -> [8, 2048, 128] f32

Sharding: one batch element per core (B == n_cores == 8). No collectives.

Per-core device kernel (bf16 inputs, f32 accumulation):
  - Host pre-transposes X -> XT [1024, 2048] (8 per-chunk DRAM->SBUF tiles so
    the PE can start as soon as chunk 0 lands) and folds 1/sqrt(S) into Wq/bq.
  - QT/KT/VT [128(e), 2048(tok)] = W^T @ XT (PE, K=1024, N=512 matmuls).
  - V [2048(key), 128] from VT via 16 PE transposes.
  - Per query block t (512 queries):
      S^T tiles [128(key), 512(q)] (PE, N=512); exp on ScalarE over 2-bank
      PSUM groups (no max-subtraction: |scores| <= ~2 by construction);
      mask folded into E by per-partition multiply with mask(key) in {0,1};
      numerator OT [128(e), 512(q)] = sum_j V_j^T @ E_j (PE, N=512, V_j
      stationary); denominator = ones^T @ (DVE pairwise tree-sum of E_j)
      (one [K=128,M=1,N=512] matmul); reciprocal on DVE, gpsimd
      partition_broadcast, final DVE multiply. The denominator/normalize
      chain is deferred one query block so the PE never waits on ScalarE/DVE.
  - Output written as OT [128, 2048]; host transposes to [2048, 128].
"""

import sys
import types

for _p in ("/opt/trn_rl_repo", "/root/.axon_site/_ro/trn_rl_repo"):
    if _p not in sys.path:
        sys.path.append(_p)

import numpy as np
import ml_dtypes

B, S, DIN, DOUT = 8, 2048, 1024, 128
NCHUNK = DIN // 128          # 8 contraction chunks
NKEY = S // 128              # 16 key chunks
QBLK = 512                   # query block (free dim of S^T / OT matmuls)
NQB = S // QBLK              # 4 query blocks
STG = 2                      # key chunks per exp group ([128, STG*512] psum)
NGRP = NKEY // STG           # 8 exp groups per query block

BF16 = ml_dtypes.bfloat16


def _build():
    import concourse.bass as bass
    import concourse.tile as tile
    from concourse import bacc, mybir
    from concourse.masks import make_identity

    f32 = mybir.dt.float32
    bf16 = mybir.dt.bfloat16
    Exp = mybir.ActivationFunctionType.Exp

    nc = bacc.Bacc("TRN2", target_bir_lowering=False, debug=False, num_devices=B)

    xt_d = nc.dram_tensor("xt", [DIN, S], bf16, kind="ExternalInput")
    wq_d = nc.dram_tensor("wq", [DIN, DOUT], bf16, kind="ExternalInput")
    wk_d = nc.dram_tensor("wk", [DIN, DOUT], bf16, kind="ExternalInput")
    wv_d = nc.dram_tensor("wv", [DIN, DOUT], bf16, kind="ExternalInput")
    bq_d = nc.dram_tensor("bq", [1, DOUT], bf16, kind="ExternalInput")
    bk_d = nc.dram_tensor("bk", [1, DOUT], bf16, kind="ExternalInput")
    bv_d = nc.dram_tensor("bv", [1, DOUT], bf16, kind="ExternalInput")
    m01_d = nc.dram_tensor("m01", [128, NKEY], f32, kind="ExternalInput")
    out_d = nc.dram_tensor("out", [DOUT, S], f32, kind="ExternalOutput")

    with tile.TileContext(nc) as tc:
        with (
            tc.tile_pool(name="persist", bufs=1) as pp,
            tc.tile_pool(name="epool", bufs=2 * NGRP) as ep,
            tc.tile_pool(name="tree", bufs=2) as tp,
            tc.tile_pool(name="normp", bufs=2) as rp,
            tc.tile_pool(name="outp", bufs=2) as op,
        ):
            xts = [pp.tile([128, S], bf16, tag=f"xt{c}", name=f"xt{c}")
                   for c in range(NCHUNK)]
            wq = pp.tile([128, NCHUNK * DOUT], bf16, tag="wq")
            wk = pp.tile([128, NCHUNK * DOUT], bf16, tag="wk")
            wv = pp.tile([128, NCHUNK * DOUT], bf16, tag="wv")
            bq = pp.tile([1, DOUT], bf16, tag="bq")
            bk = pp.tile([1, DOUT], bf16, tag="bk")
            bv = pp.tile([1, DOUT], bf16, tag="bv")
            m01 = pp.tile([128, NKEY], f32, tag="m01")
            ones = pp.tile([1, QBLK], bf16, tag="ones")
            ocol = pp.tile([128, 1], bf16, tag="ocol")
            ident = pp.tile([128, 128], bf16, tag="ident")
            qt = pp.tile([128, S], bf16, tag="qt")
            kt = pp.tile([128, S], bf16, tag="kt")
            vt = pp.tile([128, S], bf16, tag="vt")
            vn = pp.tile([128, NKEY * 128], bf16, tag="vn")

            nc.sync.dma_start(wq[:].rearrange("p (c e) -> p c e", c=NCHUNK),
                              wq_d.ap().rearrange("(c p) e -> p c e", p=128))
            nc.sync.dma_start(wk[:].rearrange("p (c e) -> p c e", c=NCHUNK),
                              wk_d.ap().rearrange("(c p) e -> p c e", p=128))
            nc.sync.dma_start(wv[:].rearrange("p (c e) -> p c e", c=NCHUNK),
                              wv_d.ap().rearrange("(c p) e -> p c e", p=128))
            nc.sync.dma_start(bq[:], bq_d.ap())
            nc.sync.dma_start(bk[:], bk_d.ap())
            nc.sync.dma_start(bv[:], bv_d.ap())
            nc.sync.dma_start(m01[:], m01_d.ap())
            nc.vector.memset(ones[:], 1.0)
            nc.vector.memset(ocol[:], 1.0)
            make_identity(nc, ident[:])

            xt3 = xt_d.ap().rearrange("(c p) m -> p c m", p=128)
            for c in range(NCHUNK):
                nc.sync.dma_start(xts[c][:], xt3[:, c, :])

            # ---- Phase A: QT / KT / VT projections ----
            with tc.tile_pool(name="ps_a", bufs=NQB, space="PSUM") as ps_a:
                for w, bias, dst, nm in ((wq, bq, qt, "q"), (wk, bk, kt, "k"),
                                         (wv, bv, vt, "v")):
                    ps = [ps_a.tile([128, QBLK], f32, tag="a", name=f"pa{nm}{t}")
                          for t in range(NQB)]
                    for c in range(NCHUNK):
                        for t in range(NQB):
                            nc.tensor.matmul(
                                ps[t][:],
                                w[:, c * DOUT:(c + 1) * DOUT],
                                xts[c][:, t * QBLK:(t + 1) * QBLK],
                                start=(c == 0), stop=False,
                            )
                    for t in range(NQB):
                        nc.tensor.matmul(ps[t][:], bias[:], ones[:],
                                         start=False, stop=True)
                    for t in range(NQB):
                        nc.vector.tensor_copy(dst[:, t * QBLK:(t + 1) * QBLK],
                                              ps[t][:])

            # ---- Phase B: V natural layout via PE transpose ----
            with tc.tile_pool(name="ps_tr", bufs=3, space="PSUM") as ps_tr:
                for k in range(NKEY):
                    ptr = ps_tr.tile([128, 128], bf16, tag="tr")
                    nc.tensor.transpose(ptr[:], vt[:, k * 128:(k + 1) * 128],
                                        ident[:])
                    nc.vector.tensor_copy(vn[:, k * 128:(k + 1) * 128], ptr[:])

            # ---- Phase C: attention ----
            with (
                tc.tile_pool(name="ps_st", bufs=2, space="PSUM") as ps_st,
                tc.tile_pool(name="ps_o", bufs=2, space="PSUM") as ps_o,
                tc.tile_pool(name="ps_m", bufs=1, space="PSUM") as ps_m,
            ):
                def finish(st):
                    t, pot, pd = st
                    rd = rp.tile([1, QBLK], f32, tag="rd", name=f"rd{t}")
                    nc.vector.reciprocal(rd[:], pd[:])
                    rdb = rp.tile([128, QBLK], f32, tag="rdb", name=f"rdb{t}")
                    nc.gpsimd.partition_broadcast(rdb[:], rd[:])
                    osb = op.tile([128, QBLK], f32, tag="osb", name=f"osb{t}")
                    nc.vector.tensor_mul(osb[:], pot[:], rdb[:])
                    nc.sync.dma_start(out_d.ap()[:, t * QBLK:(t + 1) * QBLK],
                                      osb[:])

                pending = None
                for t in range(NQB):
                    egs = []
                    for g in range(NGRP):
                        pst = ps_st.tile([128, STG * QBLK], f32, tag="st")
                        for jj in range(STG):
                            j = g * STG + jj
                            nc.tensor.matmul(
                                pst[:, jj * QBLK:(jj + 1) * QBLK],
                                kt[:, j * 128:(j + 1) * 128],
                                qt[:, t * QBLK:(t + 1) * QBLK],
                                start=True, stop=True,
                            )
                        eg = ep.tile([128, STG * QBLK], bf16, tag="e",
                                     name=f"eg{t}_{g}")
                        nc.scalar.activation(eg[:], pst[:], Exp)
                        for jj in range(STG):
                            j = g * STG + jj
                            sl = eg[:, jj * QBLK:(jj + 1) * QBLK]
                            nc.vector.tensor_scalar_mul(sl, sl, m01[:, j:j + 1])
                        egs.append(eg)

                    # numerator: OT += V_j^T @ E_j  (V_j stationary, N=512)
                    pot = ps_o.tile([128, QBLK], f32, tag="o", name=f"pot{t}")
                    for j in range(NKEY):
                        g, jj = j // STG, j % STG
                        nc.tensor.matmul(
                            pot[:],
                            vn[:, j * 128:(j + 1) * 128],
                            egs[g][:, jj * QBLK:(jj + 1) * QBLK],
                            start=(j == 0), stop=(j == NKEY - 1),
                        )

                    # denominator: pairwise DVE tree over the 16 E slices,
                    # then ones^T @ esum on PE.
                    lvl = []
                    for g in range(NGRP):
                        a = tp.tile([128, QBLK], bf16, tag=f"t1_{g % 4}",
                                    name=f"a{t}_{g}", bufs=3)
                        nc.vector.tensor_add(a[:], egs[g][:, :QBLK],
                                             egs[g][:, QBLK:])
                        lvl.append(a)
                    while len(lvl) > 1:
                        nxt = []
                        for i in range(0, len(lvl), 2):
                            a = tp.tile([128, QBLK], bf16,
                                        tag=f"t2_{len(lvl)}_{i % 2}",
                                        name=f"s{t}_{len(lvl)}_{i}", bufs=2)
                            nc.vector.tensor_add(a[:], lvl[i][:], lvl[i + 1][:])
                            nxt.append(a)
                        lvl = nxt
                    pd = ps_m.tile([1, QBLK], f32, tag="d", name=f"pd{t}")
                    nc.tensor.matmul(pd[:], ocol[:], lvl[0][:],
                                     start=True, stop=True)

                    if pending is not None:
                        finish(pending)
                    pending = (t, pot, pd)
                finish(pending)

    nc.compile()
    return nc


_NC = None


def _get_nc():
    global _NC
    if _NC is None:
        _NC = _build()
    return _NC


def _prep_in_maps(input_tensor, attention_mask, Wq, bq, Wk, bk, Wv, bv):
    scale = np.float32(1.0 / np.sqrt(np.float32(S)))
    wq_h = (np.asarray(Wq, np.float32) * scale).astype(BF16)
    wk_h = np.asarray(Wk, np.float32).astype(BF16)
    wv_h = np.asarray(Wv, np.float32).astype(BF16)
    bq_h = (np.asarray(bq, np.float32) * scale).astype(BF16).reshape(1, DOUT)
    bk_h = np.asarray(bk, np.float32).astype(BF16).reshape(1, DOUT)
    bv_h = np.asarray(bv, np.float32).astype(BF16).reshape(1, DOUT)

    x = np.asarray(input_tensor, np.float32)
    m = np.asarray(attention_mask)
    in_maps = []
    for b in range(B):
        xt_h = np.ascontiguousarray(x[b].T).astype(BF16)            # [DIN, S]
        m01_h = np.ascontiguousarray(
            m[b].astype(np.float32).reshape(NKEY, 128).T)           # [128, NKEY]
        in_maps.append({
            "xt": xt_h, "wq": wq_h, "wk": wk_h, "wv": wv_h,
            "bq": bq_h, "bk": bk_h, "bv": bv_h, "m01": m01_h,
        })
    return in_maps


def run(in_maps, trace=False, **kwargs):
    from concourse.bass_utils import run_bass_kernel_spmd

    nc = _get_nc()
    return run_bass_kernel_spmd(
        nc, in_maps, core_ids=list(range(B)), trace=trace, **kwargs
    )


def kernel(input_tensor, attention_mask, Wq, bq, Wk, bk, Wv, bv):
    in_maps = _prep_in_maps(
        input_tensor, attention_mask, Wq, bq, Wk, bk, Wv, bv)
    res = run(in_maps, trace=False)
    out = np.stack([res.results[b]["out"].T for b in range(B)])
    return np.ascontiguousarray(out.astype(np.float32))



# revision 5
# speedup vs baseline: 1.0878x; 1.0878x over previous
"""AttentionHead kernel for Trainium2, 8 NeuronCores, data-parallel over batch.

Problem (fixed shapes):
    input_tensor [8, 2048, 1024] f32, attention_mask [8, 2048] int64 (0/1),
    Wq/Wk/Wv [1024, 128] f32, bq/bk/bv [128] f32.
    out = softmax(mask(Q @ K^T / sqrt(2048))) @ V    -> [8, 2048, 128] f32

Sharding: one batch element per core (B == n_cores == 8). No collectives.

Per-core device kernel (bf16 inputs, f32 accumulation). v2 design notes:
  - DMA order: wq, xt0, wk, xt1, bcol, mcol, xt2..xt7, wv so the PE can start
    the projection pipeline as soon as wq+xt0 land (~4.5us) instead of waiting
    for the whole 4MB X transfer.
  - Pass1 computes QT and KT chunk-by-chunk (DMA paced), pass2 computes VT
    from SBUF-resident X. PSUM is managed as 4 tags x 2 banks (q01/q23/
    k01/k23); each [128,1024] f32 slot holds two accumulation groups.
  - Mask handling is OFF the exp critical path entirely:
      * numerator: V rows are zeroed for masked keys during the V transpose
        copies (tensor_scalar_mul by the 0/1 mask column) -- free.
      * denominator: 16 matmuls per query block with lhsT = mask column
        ([128,1] 0/1 bf16), accumulating sum_j m_j^T E_j into PSUM row.
    So exp output feeds the PE directly; no DVE mask muls, no DVE tree.
  - Numerator [128,512] and denominator [1,512] share one 2-bank PSUM tile.
  - Normalize: reciprocal_approx_fast (5x faster than DVE reciprocal),
    gpsimd partition_broadcast, one DVE multiply, per-block out DMA.
  - Scores tiles ([128, 2*512] = one exp group) double-buffer through tags
    k01/k23; PE program order interleaves next-block scores between
    numerator/denominator matmul pairs so ScalarE (exp) stays saturated.
"""

import sys

for _p in ("/opt/trn_rl_repo", "/root/.axon_site/_ro/trn_rl_repo"):
    if _p not in sys.path:
        sys.path.append(_p)

import numpy as np
import ml_dtypes

B, S, DIN, DOUT = 8, 2048, 1024, 128
NCHUNK = DIN // 128          # 8 contraction chunks
NKEY = S // 128              # 16 key chunks
QBLK = 512                   # query block (free dim of S^T / OT matmuls)
NQB = S // QBLK              # 4 query blocks
STG = 2                      # key chunks per exp group ([128, STG*512] psum)
NGRP = NKEY // STG           # 8 exp groups per query block

BF16 = ml_dtypes.bfloat16


def _build():
    import concourse.bass as bass
    import concourse.tile as tile
    from concourse import bacc, mybir
    from concourse.masks import make_identity

    f32 = mybir.dt.float32
    bf16 = mybir.dt.bfloat16
    Exp = mybir.ActivationFunctionType.Exp

    nc = bacc.Bacc("TRN2", target_bir_lowering=False, debug=False, num_devices=B)

    xt_d = nc.dram_tensor("xt", [DIN, S], bf16, kind="ExternalInput")
    wq_d = nc.dram_tensor("wq", [DIN, DOUT], bf16, kind="ExternalInput")
    wk_d = nc.dram_tensor("wk", [DIN, DOUT], bf16, kind="ExternalInput")
    wv_d = nc.dram_tensor("wv", [DIN, DOUT], bf16, kind="ExternalInput")
    bcol_d = nc.dram_tensor("bcol", [128, 3], f32, kind="ExternalInput")
    mcol_d = nc.dram_tensor("mcol", [128, NKEY], bf16, kind="ExternalInput")
    mcf_d = nc.dram_tensor("mcf", [128, NKEY], f32, kind="ExternalInput")
    out_d = nc.dram_tensor("out", [DOUT, S], f32, kind="ExternalOutput")

    with tile.TileContext(nc) as tc:
        with (
            tc.tile_pool(name="persist", bufs=1) as pp,
            tc.tile_pool(name="epool", bufs=4) as ep,
            tc.tile_pool(name="normp", bufs=2) as rp,
            tc.tile_pool(name="outp", bufs=2) as op,
            tc.tile_pool(name="psum", bufs=1, space="PSUM") as ps,
        ):
            xts = [pp.tile([128, S], bf16, tag=f"xt{c}", name=f"xt{c}")
                   for c in range(NCHUNK)]
            wq = pp.tile([128, NCHUNK * DOUT], bf16, tag="wq")
            wk = pp.tile([128, NCHUNK * DOUT], bf16, tag="wk")
            wv = pp.tile([128, NCHUNK * DOUT], bf16, tag="wv")
            bcol = pp.tile([128, 3], f32, tag="bcol")
            mcol = pp.tile([128, NKEY], bf16, tag="mcol")
            mcf = pp.tile([128, NKEY], f32, tag="mcf")
            ident = pp.tile([128, 128], bf16, tag="ident")
            qt = pp.tile([128, S], bf16, tag="qt")
            kt = pp.tile([128, S], bf16, tag="kt")
            vt = pp.tile([128, S], bf16, tag="vt")
            vn = pp.tile([128, NKEY * 128], bf16, tag="vn")

            # ---- DMA issue order: wq, xt0, wk, xt1, bcol, mcol, xt2.., wv
            xt3 = xt_d.ap().rearrange("(c p) m -> p c m", p=128)
            nc.sync.dma_start(wq[:].rearrange("p (c e) -> p c e", c=NCHUNK),
                              wq_d.ap().rearrange("(c p) e -> p c e", p=128))
            nc.sync.dma_start(xts[0][:], xt3[:, 0, :])
            nc.sync.dma_start(wk[:].rearrange("p (c e) -> p c e", c=NCHUNK),
                              wk_d.ap().rearrange("(c p) e -> p c e", p=128))
            nc.sync.dma_start(xts[1][:], xt3[:, 1, :])
            nc.sync.dma_start(bcol[:], bcol_d.ap())
            nc.sync.dma_start(mcol[:], mcol_d.ap())
            nc.sync.dma_start(mcf[:], mcf_d.ap())
            for c in range(2, NCHUNK):
                nc.sync.dma_start(xts[c][:], xt3[:, c, :])
            nc.sync.dma_start(wv[:].rearrange("p (c e) -> p c e", c=NCHUNK),
                              wv_d.ap().rearrange("(c p) e -> p c e", p=128))
            make_identity(nc, ident[:])

            # PSUM slots: 4 tags x [128,1024] f32 (2 banks each).
            def pslot(tag, cyc, shape=None, dtype=f32):
                return ps.tile(shape or [128, 2 * QBLK], dtype, tag=tag,
                               name=f"{tag}_c{cyc}")

            # ---- Phase A pass1: QT, KT accumulation (DMA paced) ----
            pq = [pslot("q01", 0), pslot("q23", 0)]   # q t0/t1, t2/t3
            pk = [pslot("k01", 0), pslot("k23", 0)]
            for c in range(NCHUNK):
                st, sp = (c == 0), (c == NCHUNK - 1)
                for t in range(NQB):
                    nc.tensor.matmul(
                        pq[t // 2][:, (t % 2) * QBLK:(t % 2 + 1) * QBLK],
                        wq[:, c * DOUT:(c + 1) * DOUT],
                        xts[c][:, t * QBLK:(t + 1) * QBLK],
                        start=st, stop=sp,
                    )
                for t in range(NQB):
                    nc.tensor.matmul(
                        pk[t // 2][:, (t % 2) * QBLK:(t % 2 + 1) * QBLK],
                        wk[:, c * DOUT:(c + 1) * DOUT],
                        xts[c][:, t * QBLK:(t + 1) * QBLK],
                        start=st, stop=sp,
                    )

            # PSUM -> SBUF copies with bias add (DVE). kt/qt block0 first so
            # the first scores matmuls unblock as early as possible.
            def drain(dst, src, bc, t):
                nc.vector.tensor_scalar_add(
                    dst[:, t * QBLK:(t + 1) * QBLK],
                    src[t // 2][:, (t % 2) * QBLK:(t % 2 + 1) * QBLK],
                    bcol[:, bc:bc + 1])

            drain(kt, pk, 1, 0)
            drain(qt, pq, 0, 0)
            for t in range(1, NQB):
                drain(kt, pk, 1, t)
            for t in range(1, NQB):
                drain(qt, pq, 0, t)

            # ---- Phase C state ----
            egs = {}           # (t, g) -> exp tile
            ktag = ["k01", "k23"]
            kcyc = [1, 1]
            sgi = [0]          # global scores-group index for tag alternation

            def scores(t, g):
                i = sgi[0] % 2
                pst = pslot(ktag[i], kcyc[i])
                kcyc[i] += 1
                sgi[0] += 1
                for jj in range(STG):
                    j = g * STG + jj
                    nc.tensor.matmul(
                        pst[:, jj * QBLK:(jj + 1) * QBLK],
                        kt[:, j * 128:(j + 1) * 128],
                        qt[:, t * QBLK:(t + 1) * QBLK],
                        start=True, stop=True,
                    )
                eg = ep.tile([128, STG * QBLK], bf16, tag="e",
                             name=f"eg{t}_{g}", bufs=4)
                nc.scalar.activation(eg[:], pst[:], Exp)
                egs[(t, g)] = eg

            # ---- Phase A pass2: VT (SBUF resident), interleaved with the
            # first query block's scores so ScalarE starts early.
            pv = [pslot("q01", 1), pslot("q23", 1)]
            scores(0, 0)
            scores(0, 1)
            for c in range(NCHUNK):
                st, sp = (c == 0), (c == NCHUNK - 1)
                for t in range(NQB):
                    nc.tensor.matmul(
                        pv[t // 2][:, (t % 2) * QBLK:(t % 2 + 1) * QBLK],
                        wv[:, c * DOUT:(c + 1) * DOUT],
                        xts[c][:, t * QBLK:(t + 1) * QBLK],
                        start=st, stop=sp,
                    )
                if c == 3:
                    scores(0, 2)
                    scores(0, 3)
            for t in range(NQB):
                drain(vt, pv, 2, t)
            scores(0, 4)
            scores(0, 5)

            # ---- Phase B: V natural layout via PE transpose, mask folded
            # into the PSUM->SBUF copy (zero masked key rows of V).
            qtag = ["q01", "q23"]
            qcyc = [2, 2]
            for kb in range(4):                    # 4 transposes per tile
                i = kb % 2
                ptr = pslot(qtag[i], qcyc[i], shape=[128, 4 * 128], dtype=bf16)
                qcyc[i] += 1
                for jj in range(4):
                    j = kb * 4 + jj
                    nc.tensor.transpose(ptr[:, jj * 128:(jj + 1) * 128],
                                        vt[:, j * 128:(j + 1) * 128],
                                        ident[:])
                for jj in range(4):
                    j = kb * 4 + jj
                    nc.vector.tensor_scalar_mul(
                        vn[:, j * 128:(j + 1) * 128],
                        ptr[:, jj * 128:(jj + 1) * 128],
                        mcf[:, j:j + 1])
            scores(0, 6)
            scores(0, 7)

            # ---- Phase C main loop ----
            ods = {}

            def numden(t, j):
                od = ods[t]
                g, jj = j // STG, j % STG
                eg = egs[(t, g)]
                nc.tensor.matmul(
                    od[:, 0:QBLK],
                    vn[:, j * 128:(j + 1) * 128],
                    eg[:, jj * QBLK:(jj + 1) * QBLK],
                    start=(j == 0), stop=(j == NKEY - 1),
                )
                nc.tensor.matmul(
                    od[0:1, QBLK:2 * QBLK],
                    mcol[:, j:j + 1],
                    eg[:, jj * QBLK:(jj + 1) * QBLK],
                    start=(j == 0), stop=(j == NKEY - 1),
                )

            def finish(t):
                od = ods[t]
                rd = rp.tile([1, QBLK], f32, tag="rd", name=f"rd{t}")
                nc.vector.reciprocal_approx_fast(rd[:], od[0:1, QBLK:2 * QBLK])
                rdb = rp.tile([128, QBLK], f32, tag="rdb", name=f"rdb{t}")
                nc.gpsimd.partition_broadcast(rdb[:], rd[:])
                osb = op.tile([128, QBLK], f32, tag="osb", name=f"osb{t}")
                nc.vector.tensor_mul(osb[:], od[:, 0:QBLK], rdb[:])
                nc.sync.dma_start(out_d.ap()[:, t * QBLK:(t + 1) * QBLK],
                                  osb[:])

            for t in range(NQB):
                i = t % 2
                ods[t] = pslot(qtag[i], qcyc[i])
                qcyc[i] += 1
                for g in range(NGRP):
                    if t + 1 < NQB:
                        scores(t + 1, g)
                    numden(t, STG * g)
                    numden(t, STG * g + 1)
                finish(t)

    nc.compile()
    return nc


_NC = None


def _get_nc():
    global _NC
    if _NC is None:
        _NC = _build()
    return _NC


def _prep_in_maps(input_tensor, attention_mask, Wq, bq, Wk, bk, Wv, bv):
    scale = np.float32(1.0 / np.sqrt(np.float32(S)))
    wq_h = (np.asarray(Wq, np.float32) * scale).astype(BF16)
    wk_h = np.asarray(Wk, np.float32).astype(BF16)
    wv_h = np.asarray(Wv, np.float32).astype(BF16)
    bcol_h = np.stack(
        [np.asarray(bq, np.float32) * scale,
         np.asarray(bk, np.float32),
         np.asarray(bv, np.float32)], axis=1).astype(np.float32)  # [128,3]

    x = np.asarray(input_tensor, np.float32)
    m = np.asarray(attention_mask)
    in_maps = []
    for b in range(B):
        xt_h = np.ascontiguousarray(x[b].T).astype(BF16)            # [DIN, S]
        mcf_h = np.ascontiguousarray(
            m[b].astype(np.float32).reshape(NKEY, 128).T)
        in_maps.append({
            "xt": xt_h, "wq": wq_h, "wk": wk_h, "wv": wv_h,
            "bcol": bcol_h, "mcol": mcf_h.astype(BF16), "mcf": mcf_h,
        })
    return in_maps


def run(in_maps, trace=False, **kwargs):
    from concourse.bass_utils import run_bass_kernel_spmd

    nc = _get_nc()
    return run_bass_kernel_spmd(
        nc, in_maps, core_ids=list(range(B)), trace=trace, **kwargs
    )


def kernel(input_tensor, attention_mask, Wq, bq, Wk, bk, Wv, bv):
    in_maps = _prep_in_maps(
        input_tensor, attention_mask, Wq, bq, Wk, bk, Wv, bv)
    res = run(in_maps, trace=False)
    out = np.stack([res.results[b]["out"].T for b in range(B)])
    return np.ascontiguousarray(out.astype(np.float32))
